# revision 24
# baseline (speedup 1.0000x reference)
"""Channel-Transformer WGCN layer on 8 trn2 NeuronCores.

Sharding: data-parallel over batch B=16 -> 2 batches per core; the small
[N,N]/[N] parameters are replicated. Host side does layout-only work
(slicing, transposes, dtype views); all arithmetic runs on device.

Math notes (vs reference.py):
 - The left D^-1/2 row scaling of the GCN output cancels in the final
   layer_norm (per-row scale invariance), so only the right scaling
   (dinv over the contraction index m) is applied - folded into W_A^T.
 - softmax max-subtraction is skipped: scores are O(1) for these
   normalized inputs, far from exp() overflow.
 - leaky_relu(x) = 0.01*x + relu(0.99*x) exactly (both branches).
 - Biases are folded into PE accumulation as rank-1 (K=1) matmuls,
   or into ACT eviction bias where per-partition.
 - q/k projections are fused with the score matmul per g-chunk, so
   q^T/k^T never materialize beyond one [128,512] chunk each.
"""

import sys

sys.path.insert(0, "/opt/trn_rl_repo")

from contextlib import ExitStack

import numpy as np

import concourse.bass as bass
import concourse.bacc as bacc
import concourse.mybir as mybir
import concourse.tile as tile
from concourse.alu_op_type import AluOpType
from concourse.bass_utils import run_bass_kernel_spmd

# Problem shapes (hardcoded per contract).
B, N, IN_DIM, OUT_DIM = 16, 1024, 512, 512
NCORES = 8
BL = B // NCORES  # batches per core
P = 128
HC = N // P  # 8: chunks of the node axis (h/g/m/n)
IC = IN_DIM // P  # 4: chunks of the input-feature axis
OC = OUT_DIM // P  # 4: chunks of the channel axis (x/y)
O = OUT_DIM
NEG = 0.01
EPS = 1e-5
CH_SCALE = float(N) ** -0.5

dt = mybir.dt
F32 = dt.float32
BF16 = dt.bfloat16
AF = mybir.ActivationFunctionType
ALU = AluOpType

# Matmul operand dtype: float32r runs the PE at 1 cyc/row (vs 4 for
# float32) for moving dims >= 256. Bitwise-identical storage to fp32.
MM_DT = dt.float32


def _r(ap):
    return ap.bitcast(MM_DT)


def build_program(debug=False):
    nc = bacc.Bacc("TRN2", target_bir_lowering=False)

    # Inputs (per core). nfT/adjT are host-transposed slices.
    nfT_h = nc.declare_dram_parameter("nfT", [BL, IN_DIM, N], F32, isOutput=False)
    adjT_h = nc.declare_dram_parameter("adjT", [BL, N, N], F32, isOutput=False)
    ewT_h = nc.declare_dram_parameter("ewT", [N, N], F32, isOutput=False)
    lwT_h = nc.declare_dram_parameter("lwT", [IN_DIM, O], F32, isOutput=False)
    qwT_h = nc.declare_dram_parameter("qwT", [N, N], F32, isOutput=False)
    kwT_h = nc.declare_dram_parameter("kwT", [N, N], F32, isOutput=False)
    vwT_h = nc.declare_dram_parameter("vwT", [N, N], F32, isOutput=False)
    lb_h = nc.declare_dram_parameter("lb", [O], F32, isOutput=False)
    qb_h = nc.declare_dram_parameter("qb", [N], F32, isOutput=False)
    kb_h = nc.declare_dram_parameter("kb", [N], F32, isOutput=False)
    vb_h = nc.declare_dram_parameter("vb", [N], F32, isOutput=False)
    n1w_h = nc.declare_dram_parameter("n1w", [N], F32, isOutput=False)
    n1b_h = nc.declare_dram_parameter("n1b", [N], F32, isOutput=False)
    n2w_h = nc.declare_dram_parameter("n2w", [O], F32, isOutput=False)
    n2b_h = nc.declare_dram_parameter("n2b", [O], F32, isOutput=False)
    out_h = nc.declare_dram_parameter("out", [BL, N, O], F32, isOutput=True)
    dbg = {}
    if debug:
        for nm, shp in [
            ("dX1", [BL, P, HC, O]), ("dTxN", [BL, P, HC, O]),
            ("dET", [BL, P, OC, O]), ("dvT", [BL, P, OC, N]),
            ("dX2", [BL, P, HC, O]), ("dwa", [BL, P, HC, N]),
            ("dR", [BL, P, HC, O]),
        ]:
            dbg[nm] = nc.declare_dram_parameter(nm, shp, F32, isOutput=True)

    with tile.TileContext(nc) as tc, ExitStack() as ctx:
        singles = ctx.enter_context(tc.tile_pool(name="singles", bufs=1))

        def load_chunked(dram_ap, cchunks, free, tg):
            t = singles.tile([P, cchunks, free], F32, name=tg, tag=tg)
            nc.sync.dma_start(out=t, in_=dram_ap.rearrange("(c p) f -> p c f", p=P))
            return t

        def load_col(vec_h, chunks, tg):
            t = singles.tile([P, chunks], F32, name=tg, tag=tg)
            nc.sync.dma_start(out=t, in_=vec_h[:].rearrange("(c p) -> p c", p=P))
            return t

        def load_row(vec_h, n, tg):
            t = singles.tile([1, n], F32, name=tg, tag=tg)
            nc.sync.dma_start(out=t, in_=vec_h[:].rearrange("(a n) -> a n", a=1))
            return t

        lwT = load_chunked(lwT_h[:], IC, O, "w_lwT")
        qwT = load_chunked(qwT_h[:], HC, N, "w_qwT")
        kwT = load_chunked(kwT_h[:], HC, N, "w_kwT")
        vwT = load_chunked(vwT_h[:], HC, N, "w_vwT")

        lb_row = load_row(lb_h, O, "w_lb")
        vb_row = load_row(vb_h, N, "w_vb")
        qb_col = load_col(qb_h, HC, "w_qb")
        kb_col = load_col(kb_h, HC, "w_kb")
        n1w_col = load_col(n1w_h, HC, "w_n1w")
        n1b_col = load_col(n1b_h, HC, "w_n1b")
        n2w_row = load_row(n2w_h, O, "w_n2w")
        n2b_row = load_row(n2b_h, O, "w_n2b")

        ones_col = singles.tile([P, 1], F32)
        nc.vector.memset(ones_col, 1.0)
        ones_row = singles.tile([1, P], F32)
        nc.vector.memset(ones_row, 1.0)
        eps_col = singles.tile([P, 1], F32)
        nc.vector.memset(eps_col, EPS)

        # global pools
        mmps = ctx.enter_context(tc.tile_pool(name="mmps", bufs=3, space="PSUM"))
        p_tmp = ctx.enter_context(tc.tile_pool(name="p_tmp", bufs=3))
        p_vec = ctx.enter_context(tc.tile_pool(name="p_vec", bufs=4))

        for b in range(BL):
            if True:
                # ---------- Phase 1: X1 = leaky(nf @ lw^T + lb)  [h, o]
                p_X2 = tc.alloc_tile_pool(name=f"pX2_{b}", bufs=1)
                p_TxN = tc.alloc_tile_pool(name=f"pTxN_{b}", bufs=1)
                p1 = tc.alloc_tile_pool(name=f"p1_{b}", bufs=1)
                nfT = p1.tile([P, IC, N], F32, name=f"nfT{b}", tag="nfT")
                nc.sync.dma_start(
                    out=nfT, in_=nfT_h[b].rearrange("(c p) n -> p c n", p=P)
                )
                X1 = p1.tile([P, HC, O], F32, name=f"X1{b}", tag="X1")
                for hs in range(HC):
                    ps = mmps.tile([P, O], F32, tag="mm")
                    for icc in range(IC):
                        nc.tensor.matmul(
                            ps,
                            lhsT=_r(nfT[:, icc, hs * P : (hs + 1) * P]),
                            rhs=_r(lwT[:, icc, :]),
                            start=(icc == 0),
                            stop=False,
                        )
                    nc.tensor.matmul(  # + lb (rank-1 bias)
                        ps, lhsT=_r(ones_row), rhs=_r(lb_row), start=False, stop=True
                    )
                    tr = p_tmp.tile([P, O], F32, tag="tmp")
                    nc.scalar.activation(tr, ps, AF.Relu, scale=0.99)
                    nc.vector.scalar_tensor_tensor(
                        out=X1[:, hs, :], in0=ps, scalar=NEG, in1=tr,
                        op0=ALU.mult, op1=ALU.add,
                    )

                if debug:
                    nc.sync.dma_start(out=dbg["dX1"][b], in_=X1)
                # ---------- Phase 2: LN1 over h (partitions) via PE-ones
                with tc.tile_pool(name=f"st_{b}", bufs=2, space="PSUM") as statps, \
                     tc.tile_pool(name=f"rp_{b}", bufs=2, space="PSUM") as repps:
                    s_ps = statps.tile([1, O], F32, tag="st")
                    s2_ps = statps.tile([1, O], F32, tag="st")
                    for hs in range(HC):
                        nc.tensor.matmul(
                            s_ps, lhsT=_r(ones_col), rhs=_r(X1[:, hs, :]),
                            start=(hs == 0), stop=(hs == HC - 1),
                        )
                    for hs in range(HC):
                        xsq = p_tmp.tile([P, O], F32, tag="tmp")
                        nc.scalar.activation(xsq, X1[:, hs, :], AF.Square)
                        nc.tensor.matmul(
                            s2_ps, lhsT=_r(ones_col), rhs=_r(xsq),
                            start=(hs == 0), stop=(hs == HC - 1),
                        )
                    mu = p_vec.tile([1, O], F32, tag="vrow")
                    nc.vector.tensor_scalar(
                        out=mu, in0=s_ps, scalar1=1.0 / N, scalar2=None, op0=ALU.mult
                    )
                    musq = p_vec.tile([1, O], F32, tag="vrow")
                    nc.vector.tensor_mul(musq, mu, mu)
                    var = p_vec.tile([1, O], F32, tag="vrow")
                    nc.vector.scalar_tensor_tensor(
                        out=var, in0=s2_ps, scalar=1.0 / N, in1=musq,
                        op0=ALU.mult, op1=ALU.subtract,
                    )
                    sd = p_vec.tile([1, O], F32, tag="vrow")
                    nc.scalar.activation(sd, var, AF.Sqrt, bias=eps_col[0:1, :])
                    a_row = p_vec.tile([1, O], F32, tag="vrow")
                    nc.vector.reciprocal(a_row, sd)
                    b_row = p_vec.tile([1, O], F32, tag="vrow")
                    nc.vector.scalar_tensor_tensor(
                        out=b_row, in0=mu, scalar=-1.0, in1=a_row,
                        op0=ALU.mult, op1=ALU.mult,
                    )
                    a_rep = repps.tile([P, O], F32, tag="rep")
                    nc.tensor.matmul(
                        a_rep, lhsT=_r(ones_row), rhs=_r(a_row), start=True, stop=True
                    )
                    b_rep = repps.tile([P, O], F32, tag="rep")
                    nc.tensor.matmul(
                        b_rep, lhsT=_r(ones_row), rhs=_r(b_row), start=True, stop=True
                    )

                    # TxN = ((X1 - mu)*rstd) * n1w + n1b
                    TxN = p_TxN.tile([P, HC, O], F32, name=f"TxN{b}", tag="TxN")
                    for hs in range(HC):
                        t = p_tmp.tile([P, O], F32, tag="tmp")
                        nc.vector.tensor_mul(t, X1[:, hs, :], a_rep)
                        z = p_tmp.tile([P, O], F32, tag="tmp")
                        nc.vector.tensor_add(z, t, b_rep)
                        nc.vector.tensor_scalar(
                            out=TxN[:, hs, :], in0=z,
                            scalar1=n1w_col[:, hs : hs + 1],
                            scalar2=n1b_col[:, hs : hs + 1],
                            op0=ALU.mult, op1=ALU.add,
                        )
                if debug:
                    nc.sync.dma_start(out=dbg["dTxN"][b], in_=TxN)
                p1.release()

                # ---------- Phase 3-6: attention
                pA = tc.alloc_tile_pool(name=f"pA_{b}", bufs=1)
                E_T = pA.tile([P, OC, O], F32, name=f"ET{b}", tag="ET")
                v_T = pA.tile([P, OC, N], F32, name=f"vT{b}", tag="vT")

                # fused q/k projection + scores S^T, accumulated per g-chunk
                with tc.tile_pool(name=f"qk_{b}", bufs=2) as p_qk, \
                     tc.tile_pool(name=f"sps_{b}", bufs=1, space="PSUM") as sps:
                    S_ps = [sps.tile([P, O], F32, name=f"S{b}_{ys}", tag=f"S{ys}") for ys in range(OC)]
                    for gc in range(HC):
                        q_ps = mmps.tile([P, O], F32, tag="mm")
                        for hcc in range(HC):
                            nc.tensor.matmul(
                                q_ps,
                                lhsT=_r(qwT[:, hcc, gc * P : (gc + 1) * P]),
                                rhs=_r(TxN[:, hcc, :]),
                                start=(hcc == 0),
                                stop=(hcc == HC - 1),
                            )
                        q_gc = p_qk.tile([P, O], F32, tag="qgc")
                        nc.scalar.activation(
                            q_gc, q_ps, AF.Identity, bias=qb_col[:, gc : gc + 1]
                        )
                        k_ps = mmps.tile([P, O], F32, tag="mm")
                        for hcc in range(HC):
                            nc.tensor.matmul(
                                k_ps,
                                lhsT=_r(kwT[:, hcc, gc * P : (gc + 1) * P]),
                                rhs=_r(TxN[:, hcc, :]),
                                start=(hcc == 0),
                                stop=(hcc == HC - 1),
                            )
                        k_gc = p_qk.tile([P, O], F32, tag="kgc")
                        nc.scalar.activation(
                            k_gc, k_ps, AF.Identity, bias=kb_col[:, gc : gc + 1]
                        )
                        for ys in range(OC):
                            nc.tensor.matmul(
                                S_ps[ys],
                                lhsT=_r(k_gc[:, ys * P : (ys + 1) * P]),
                                rhs=_r(q_gc),
                                start=(gc == 0),
                                stop=(gc == HC - 1),
                            )
                    for ys in range(OC):
                        nc.scalar.activation(
                            E_T[:, ys, :], S_ps[ys], AF.Exp, scale=CH_SCALE
                        )

                # v projection (channel-major) + bias + leaky
                for ys in range(OC):
                    for gh in range(2):
                        fs = slice(gh * O, (gh + 1) * O)
                        ps = mmps.tile([P, O], F32, tag="mm")
                        for hcc in range(HC):
                            nc.tensor.matmul(
                                ps,
                                lhsT=_r(TxN[:, hcc, ys * P : (ys + 1) * P]),
                                rhs=_r(vwT[:, hcc, fs]),
                                start=(hcc == 0),
                                stop=False,
                            )
                        nc.tensor.matmul(  # + vb over the free axis
                            ps, lhsT=_r(ones_row), rhs=_r(vb_row[:, fs]),
                            start=False, stop=True,
                        )
                        tr = p_tmp.tile([P, O], F32, tag="tmp")
                        nc.scalar.activation(tr, ps, AF.Relu, scale=0.99)
                        nc.vector.scalar_tensor_tensor(
                            out=v_T[:, ys, fs], in0=ps, scalar=NEG, in1=tr,
                            op0=ALU.mult, op1=ALU.add,
                        )

                # att = E / colsum(E)
                with tc.tile_pool(name=f"sm_{b}", bufs=1, space="PSUM") as smps:
                    sig_ps = smps.tile([1, O], F32, tag="sm")
                    for ys in range(OC):
                        nc.tensor.matmul(
                            sig_ps, lhsT=_r(ones_col), rhs=_r(E_T[:, ys, :]),
                            start=(ys == 0), stop=(ys == OC - 1),
                        )
                    rho = p_vec.tile([1, O], F32, tag="vrow")
                    nc.vector.reciprocal(rho, sig_ps)
                    rho_rep = smps.tile([P, O], F32, tag="smr")
                    nc.tensor.matmul(
                        rho_rep, lhsT=_r(ones_row), rhs=_r(rho), start=True, stop=True
                    )
                    for ys in range(OC):
                        nc.vector.tensor_mul(E_T[:, ys, :], E_T[:, ys, :], rho_rep)

                if debug:
                    nc.sync.dma_start(out=dbg["dET"][b], in_=E_T)
                    nc.sync.dma_start(out=dbg["dvT"][b], in_=v_T)
                # ---------- Phase 8a: X2 = TxN + att @ v   (bf16, [m, o])
                X2 = p_X2.tile([P, HC, O], F32, name=f"X2{b}", tag="X2")
                for gs in range(HC):
                    ps = mmps.tile([P, O], F32, tag="mm")
                    for ys in range(OC):
                        nc.tensor.matmul(
                            ps,
                            lhsT=_r(v_T[:, ys, gs * P : (gs + 1) * P]),
                            rhs=_r(E_T[:, ys, :]),
                            start=(ys == 0),
                            stop=(ys == OC - 1),
                        )
                    nc.vector.tensor_add(X2[:, gs, :], TxN[:, gs, :], ps)
                if debug:
                    nc.sync.dma_start(out=dbg["dX2"][b], in_=X2)
                pA.release()
                p_TxN.release()

                # ---------- Phase 7: wa = dinv[m] * adj^T * sigmoid(ew^T) (bf16)
                # d[m] = sum_j W_A[m, j] is a PARTITION-axis sum of wa's
                # free columns: ones-matmuls per m-slice into one [P, HC]
                # PSUM tile (column per slice), then scale wa chunks.
                p_wa = tc.alloc_tile_pool(name=f"pwa_{b}", bufs=1)
                wa = p_wa.tile([P, HC, N], F32, name=f"wa{b}", tag="wa")
                with tc.tile_pool(name=f"ew_{b}", bufs=2) as p_ew:
                    for mc in range(HC):
                        for nh in range(2):
                            fs = slice(nh * O, (nh + 1) * O)
                            adjc = p_ew.tile([P, O], F32, tag="adjc")
                            nc.sync.dma_start(
                                out=adjc,
                                in_=adjT_h[b].rearrange("(c p) n -> p c n", p=P)[
                                    :, mc, fs
                                ],
                            )
                            sg = p_ew.tile([P, O], F32, tag="sig")
                            nc.sync.dma_start(
                                out=sg,
                                in_=ewT_h[:].rearrange("(c p) n -> p c n", p=P)[
                                    :, mc, fs
                                ],
                            )
                            sgs = p_ew.tile([P, O], F32, tag="sig")
                            nc.scalar.activation(sgs, sg, AF.Sigmoid)
                            nc.vector.scalar_tensor_tensor(
                                out=wa[:, mc, fs], in0=adjc, scalar=1.0, in1=sgs,
                                op0=ALU.mult, op1=ALU.mult,
                            )
                dinv = p_vec.tile([P, HC], F32, tag="dst")
                with tc.tile_pool(name=f"dps_{b}", bufs=1, space="PSUM") as dps:
                    dp = dps.tile([P, HC], F32, tag="dp")
                    for s in range(HC):
                        for ic in range(HC):
                            nc.tensor.matmul(
                                dp[:, s : s + 1],
                                lhsT=_r(wa[:, ic, s * P : (s + 1) * P]),
                                rhs=_r(ones_col),
                                start=(ic == 0),
                                stop=(ic == HC - 1),
                            )
                    dsq = p_vec.tile([P, HC], F32, tag="dst")
                    nc.scalar.activation(dsq, dp, AF.Sqrt)
                    nc.vector.reciprocal(dinv, dsq)
                    for s in range(HC):
                        nc.vector.tensor_scalar(
                            out=wa[:, s, :], in0=wa[:, s, :],
                            scalar1=dinv[:, s : s + 1], scalar2=None, op0=ALU.mult,
                        )

                if debug:
                    for mc in range(HC):
                        nc.sync.dma_start(out=dbg["dwa"][b][:, mc, :], in_=wa[:, mc, :])
                # ---------- Phase 9: R = wa.T @ X2; LN2; store
                p_o = tc.alloc_tile_pool(name=f"po_{b}", bufs=2)
                n2ps = tc.alloc_tile_pool(name=f"n2_{b}", bufs=2, space="PSUM")
                n2w_rep = n2ps.tile([P, O], F32, tag="n2")
                nc.tensor.matmul(
                    n2w_rep, lhsT=_r(ones_row), rhs=_r(n2w_row), start=True, stop=True
                )
                n2b_rep = n2ps.tile([P, O], F32, tag="n2")
                nc.tensor.matmul(
                    n2b_rep, lhsT=_r(ones_row), rhs=_r(n2b_row), start=True, stop=True
                )
                for ns in range(HC):
                    ps = mmps.tile([P, O], F32, tag="mm")
                    for mc in range(HC):
                        nc.tensor.matmul(
                            ps,
                            lhsT=_r(wa[:, mc, ns * P : (ns + 1) * P]),
                            rhs=_r(X2[:, mc, :]),
                            start=(mc == 0),
                            stop=(mc == HC - 1),
                        )
                    R = p_o.tile([P, O], F32, tag="R")
                    sR = p_vec.tile([P, 1], F32, tag="ln2")
                    nc.scalar.activation(R, ps, AF.Identity, accum_out=sR)
                    if debug:
                        nc.sync.dma_start(out=dbg["dR"][b][:, ns, :], in_=R)
                    rsq = p_tmp.tile([P, O], F32, tag="tmp")
                    s2R = p_vec.tile([P, 1], F32, tag="ln2")
                    nc.scalar.activation(rsq, R, AF.Square, accum_out=s2R)
                    mu2 = p_vec.tile([P, 1], F32, tag="ln2")
                    nc.vector.tensor_scalar(
                        out=mu2, in0=sR, scalar1=1.0 / O, scalar2=None, op0=ALU.mult
                    )
                    musq2 = p_vec.tile([P, 1], F32, tag="ln2")
                    nc.vector.tensor_mul(musq2, mu2, mu2)
                    var2 = p_vec.tile([P, 1], F32, tag="ln2")
                    nc.vector.scalar_tensor_tensor(
                        out=var2, in0=s2R, scalar=1.0 / O, in1=musq2,
                        op0=ALU.mult, op1=ALU.subtract,
                    )
                    # reference normalizes x~ = dinv[n]*R; replicate its
                    # eps behavior: rstd_x = 1/sqrt(dinv^2*var + eps), and
                    # scale (R - mu) by dinv*rstd_x.
                    dv = dinv[:, ns : ns + 1]
                    vx = p_vec.tile([P, 1], F32, tag="ln2")
                    nc.vector.tensor_scalar(
                        out=vx, in0=var2, scalar1=dv, scalar2=dv,
                        op0=ALU.mult, op1=ALU.mult,
                    )
                    sd2 = p_vec.tile([P, 1], F32, tag="ln2")
                    nc.scalar.activation(sd2, vx, AF.Sqrt, bias=eps_col)
                    rstd2 = p_vec.tile([P, 1], F32, tag="ln2")
                    nc.vector.reciprocal(rstd2, sd2)
                    sc2 = p_vec.tile([P, 1], F32, tag="ln2")
                    nc.vector.tensor_mul(sc2, rstd2, dv)
                    xh = p_tmp.tile([P, O], F32, tag="tmp")
                    nc.vector.tensor_scalar(
                        out=xh, in0=R, scalar1=mu2, scalar2=sc2,
                        op0=ALU.subtract, op1=ALU.mult,
                    )
                    w1 = p_tmp.tile([P, O], F32, tag="tmp")
                    nc.vector.tensor_mul(w1, xh, n2w_rep)
                    of = p_o.tile([P, O], F32, tag="out")
                    nc.vector.tensor_add(of, w1, n2b_rep)
                    nc.sync.dma_start(out=out_h[b][ns * P : (ns + 1) * P, :], in_=of)
                n2ps.release()
                p_o.release()
                p_wa.release()
                p_X2.release()

    nc.compile()
    return nc


_NC_CACHE = None


def _get_nc():
    global _NC_CACHE
    if _NC_CACHE is None:
        _NC_CACHE = build_program()
    return _NC_CACHE


def make_in_maps(
    node_feats, adj_matrix, linear_w, linear_b, q_w, q_b, k_w, k_b, v_w, v_b,
    norm1_w, norm1_b, norm2_w, norm2_b, edge_weight,
):
    f = np.float32
    nfT = np.ascontiguousarray(np.swapaxes(np.asarray(node_feats, f), 1, 2))
    adjT = np.ascontiguousarray(np.swapaxes(np.asarray(adj_matrix, f), 1, 2))
    shared = {
        "ewT": np.ascontiguousarray(np.asarray(edge_weight, f).T),
        "lwT": np.ascontiguousarray(np.asarray(linear_w, f).T),
        "qwT": np.ascontiguousarray(np.asarray(q_w, f).T),
        "kwT": np.ascontiguousarray(np.asarray(k_w, f).T),
        "vwT": np.ascontiguousarray(np.asarray(v_w, f).T),
        "lb": np.asarray(linear_b, f),
        "qb": np.asarray(q_b, f),
        "kb": np.asarray(k_b, f),
        "vb": np.asarray(v_b, f),
        "n1w": np.asarray(norm1_w, f),
        "n1b": np.asarray(norm1_b, f),
        "n2w": np.asarray(norm2_w, f),
        "n2b": np.asarray(norm2_b, f),
    }
    in_maps = []
    for c in range(NCORES):
        m = dict(shared)
        m["nfT"] = np.ascontiguousarray(nfT[c * BL : (c + 1) * BL])
        m["adjT"] = np.ascontiguousarray(adjT[c * BL : (c + 1) * BL])
        in_maps.append(m)
    return in_maps


def kernel(**inputs) -> np.ndarray:
    nc = _get_nc()
    in_maps = make_in_maps(**inputs)
    res = run_bass_kernel_spmd(nc, in_maps, list(range(NCORES)))
    outs = [res.results[c]["out"] for c in range(NCORES)]
    return np.concatenate(outs, axis=0).astype(np.float32)
